# revision 3
# baseline (speedup 1.0000x reference)
"""Trainium2 Bass kernel for the SSIM+KLDiv nn_KLD problem.

Contract: kernel(**inputs) takes FULL unsharded inputs (img1, img2, window:
numpy arrays) and returns the FULL output (scalar float32), distributing work
across 8 NeuronCores internally.

Math (matching reference.py):
  mu1 = conv(img1), mu2 = conv(img2)  [depthwise 11x11 gaussian, 'same' pad]
  sigma terms from conv(img1^2), conv(img2^2), conv(img1*img2)
  ssim = mean of per-pixel SSIM map
  kl from per-row histograms -> softmax -> KLDiv (only used if ssim > 0.75)
  out = kl + 1 - ssim if ssim > 0.75 else 1 - ssim

Device strategy (per core, 32 image pairs):
  planes s=x+y, d=x-y, s^2, d^2 (bf16)
  H-conv on PE with the plane as the stationary operand -> transposed
  intermediate in PSUM; W-conv on PE with shared banded stationary.
  Conv linearity gives mu1+-mu2 = conv(s|d), conv(xy)=(conv(s^2)-conv(d^2))/4,
  conv(x^2+y^2)=(conv(s^2)+conv(d^2))/2.
  Pointwise SSIM on ACT (PSUM evac fused with Square/scale) + DVE
  (affine_then_add chains, scalar_tensor_tensor with fused accum reduction).
  Host: sum partials across cores, final scalar combine.
"""

import sys

sys.path.insert(0, "/opt/trn_rl_repo")

import math

import numpy as np

import concourse.bass as bass  # noqa: F401  (bass types used via bacc/tile)
import concourse.tile as tile
from concourse import bacc, mybir
from concourse.bass_utils import run_bass_kernel_spmd

# Problem constants (hardcoded per the harness contract).
B, C, H, W = 256, 1, 192, 256
NCORES = 8
PPC = B // NCORES  # image pairs per core
WS = 11
SIGMA = 1.5
NBIN = 1000
C1 = 0.01**2
C2 = 0.03**2
HHI, HLO = 128, H - 128  # h-partition split (128 + 64)
WHALF = 128  # w split (2 x 128)

F32 = mybir.dt.float32
BF16 = mybir.dt.bfloat16

_CACHE = {}


def _gauss_taps():
    g = np.array(
        [math.exp(-((i - WS // 2) ** 2) / (2.0 * SIGMA**2)) for i in range(WS)],
        dtype=np.float64,
    )
    g = g / g.sum()
    return g.astype(np.float32)


def _make_bands(g):
    """Banded 1-D conv matrices.

    A[h, h'] = g[h - h' + 5]   (H-conv: out[h'] = sum_h A[h,h'] x[h])
    Bm[w, w'] = g[w - w' + 5]  (W-conv)
    """
    A = np.zeros((H, H), dtype=np.float32)
    for h in range(H):
        for hp in range(max(0, h - 5), min(H, h + 6)):
            A[h, hp] = g[h - hp + 5]
    Bm = np.zeros((W, W), dtype=np.float32)
    for w in range(W):
        for wp in range(max(0, w - 5), min(W, w + 6)):
            Bm[w, wp] = g[w - wp + 5]
    bf = np.float32  # cast to bf16 happens via ml_dtypes below
    import ml_dtypes

    to_bf16 = lambda a: a.astype(ml_dtypes.bfloat16)
    return (
        to_bf16(A[0:HHI, :]),  # bandA_hi [128, 192]
        to_bf16(A[HHI:H, :]),  # bandA_lo [64, 192]
        to_bf16(Bm[0:WHALF, :]),  # bandB0  [128, 256]
        to_bf16(Bm[WHALF:W, :]),  # bandB1  [128, 256]
    )


def _build_nc():
    """Build + finalize the per-core Bass program (same program on all 8)."""
    nc = bacc.Bacc(None, target_bir_lowering=False, debug=False)

    x_in = nc.dram_tensor("img1", [PPC, H, W], F32, kind="ExternalInput")
    y_in = nc.dram_tensor("img2", [PPC, H, W], F32, kind="ExternalInput")
    bandA_hi = nc.dram_tensor("bandA_hi", [HHI, H], BF16, kind="ExternalInput")
    bandA_lo = nc.dram_tensor("bandA_lo", [HLO, H], BF16, kind="ExternalInput")
    bandB0 = nc.dram_tensor("bandB0", [WHALF, W], BF16, kind="ExternalInput")
    bandB1 = nc.dram_tensor("bandB1", [WHALF, W], BF16, kind="ExternalInput")
    partials_out = nc.dram_tensor("partials", [128, 1], F32, kind="ExternalOutput")

    SQH = math.sqrt(0.5)
    PLANES = 4  # s, d, s2, d2

    with tile.TileContext(nc) as tc:
        with (
            tc.tile_pool(name="consts", bufs=1) as consts,
            tc.tile_pool(name="inp", bufs=3) as inp,
            tc.tile_pool(name="planes", bufs=3) as planes_pool,
            tc.tile_pool(name="zt", bufs=6) as zt_pool,
            tc.tile_pool(name="pw", bufs=3) as pw,
            tc.tile_pool(name="acc", bufs=1) as accp,
            tc.tile_pool(name="hpsum", bufs=2, space="PSUM") as hpsum,
            tc.tile_pool(name="wpsum", bufs=1, space="PSUM") as wpsum,
        ):
            # Constants into SBUF.
            A_hi = consts.tile([HHI, H], BF16)
            nc.gpsimd.dma_start(out=A_hi, in_=bandA_hi[:, :])
            A_lo = consts.tile([HLO, H], BF16)
            nc.gpsimd.dma_start(out=A_lo, in_=bandA_lo[:, :])
            B0 = consts.tile([WHALF, W], BF16)
            nc.gpsimd.dma_start(out=B0, in_=bandB0[:, :])
            B1 = consts.tile([WHALF, W], BF16)
            nc.gpsimd.dma_start(out=B1, in_=bandB1[:, :])

            partials = accp.tile([128, 2 * PPC], F32)
            nc.vector.memset(partials, 0.0)

            for p in range(PPC):
                # ---- load pair (fp32 -> bf16 cast during SWDGE DMA) ----
                xh = inp.tile([HHI, W], BF16, tag="xh")
                nc.gpsimd.dma_start(out=xh, in_=x_in[p, 0:HHI, :])
                xl = inp.tile([HLO, W], BF16, tag="xl")
                nc.gpsimd.dma_start(out=xl, in_=x_in[p, HHI:H, :])
                yh = inp.tile([HHI, W], BF16, tag="yh")
                nc.gpsimd.dma_start(out=yh, in_=y_in[p, 0:HHI, :])
                yl = inp.tile([HLO, W], BF16, tag="yl")
                nc.gpsimd.dma_start(out=yl, in_=y_in[p, HHI:H, :])

                # ---- stage2 planes (bf16): s, d, s2, d2 (hi/lo h-chunks) ----
                sh = planes_pool.tile([HHI, W], BF16, tag="sh")
                nc.vector.tensor_add(sh, xh, yh)
                sl = planes_pool.tile([HLO, W], BF16, tag="sl")
                nc.vector.tensor_add(sl, xl, yl)
                dh = planes_pool.tile([HHI, W], BF16, tag="dh")
                nc.vector.tensor_sub(dh, xh, yh)
                dl = planes_pool.tile([HLO, W], BF16, tag="dl")
                nc.vector.tensor_sub(dl, xl, yl)
                s2h = planes_pool.tile([HHI, W], BF16, tag="s2h")
                nc.vector.tensor_mul(s2h, sh, sh)
                s2l = planes_pool.tile([HLO, W], BF16, tag="s2l")
                nc.vector.tensor_mul(s2l, sl, sl)
                d2h = planes_pool.tile([HHI, W], BF16, tag="d2h")
                nc.vector.tensor_mul(d2h, dh, dh)
                d2l = planes_pool.tile([HLO, W], BF16, tag="d2l")
                nc.vector.tensor_mul(d2l, dl, dl)

                plane_tiles = [(sh, sl), (dh, dl), (s2h, s2l), (d2h, d2l)]

                # ---- H-conv (PE, plane stationary) -> transposed ZT in SBUF ----
                # ZT[w, h'] = sum_h plane[h, w] * A[h, h']
                zts = []  # per plane: (zt0, zt1) bf16 SBUF [128, H]
                for pi, (ph, pl) in enumerate(plane_tiles):
                    zt_pair = []
                    for m in range(2):  # w half
                        zpsum = hpsum.tile([WHALF, H], F32, tag="zpsum")
                        nc.tensor.matmul(
                            zpsum,
                            ph[:, m * WHALF : (m + 1) * WHALF],
                            A_hi[:, :],
                            start=True,
                            stop=False,
                        )
                        nc.tensor.matmul(
                            zpsum,
                            pl[:, m * WHALF : (m + 1) * WHALF],
                            A_lo[:, :],
                            start=False,
                            stop=True,
                        )
                        zsb = zt_pool.tile([WHALF, H], BF16, tag=f"zsb{pi}{m}")
                        nc.scalar.copy(out=zsb, in_=zpsum)
                        zt_pair.append(zsb)
                    zts.append(zt_pair)

                # ---- W-conv (PE, banded stationary) + pointwise per w'-half ----
                for m in range(2):  # output w' half
                    vout = []  # psum tiles [128, H] for planes s,d,s2,d2
                    for pi in range(PLANES):
                        zt0, zt1 = zts[pi]
                        vp = wpsum.tile([WHALF, H], F32, tag=f"vp{pi}")
                        if m == 0:
                            # w' 0:128 <- K = w 0:133
                            nc.tensor.matmul(
                                vp, B0[:, 0:WHALF], zt0[:, :], start=True, stop=False
                            )
                            nc.tensor.matmul(
                                vp,
                                B1[0:5, 0:WHALF],
                                zt1[0:5, :],
                                start=False,
                                stop=True,
                            )
                        else:
                            # w' 128:256 <- K = w 123:256. Partition slices
                            # must start at base 0/32/64, so take w 64:128
                            # (rows 64:123 of the band are zero -> harmless).
                            nc.tensor.matmul(
                                vp,
                                B0[64:WHALF, WHALF:W],
                                zt0[64:WHALF, :],
                                start=True,
                                stop=False,
                            )
                            nc.tensor.matmul(
                                vp, B1[:, WHALF:W], zt1[:, :], start=False, stop=True
                            )
                        vout.append(vp)

                    mS, mQ, A2, B2 = vout  # conv(s), conv(d), conv(s^2), conv(d^2)

                    # ---- pointwise SSIM on this [128, H] tile ----
                    # ACT evacs (PSUM -> SBUF fp32) with fused transforms
                    Ssq = pw.tile([WHALF, H], F32, tag="Ssq")
                    nc.scalar.activation(
                        out=Ssq, in_=mS, func=mybir.ActivationFunctionType.Square,
                        scale=SQH,
                    )  # (muS/sqrt2)^2 = (mu1+mu2)^2/2
                    Qsq = pw.tile([WHALF, H], F32, tag="Qsq")
                    nc.scalar.activation(
                        out=Qsq, in_=mQ, func=mybir.ActivationFunctionType.Square,
                        scale=SQH,
                    )
                    As = pw.tile([WHALF, H], F32, tag="As")
                    nc.scalar.activation(
                        out=As, in_=A2, func=mybir.ActivationFunctionType.Copy,
                        scale=0.5, bias=C1 + C2,
                    )  # conv(x2+y2) + sigma-combination constant
                    Bs = pw.tile([WHALF, H], F32, tag="Bs")
                    nc.scalar.activation(
                        out=Bs, in_=B2, func=mybir.ActivationFunctionType.Copy,
                        scale=0.5,
                    )

                    # DVE chain
                    num1 = pw.tile([WHALF, H], F32, tag="num1")
                    nc.vector.affine_then_add(num1, Qsq, Ssq, -1.0, C1)  # 2P+C1
                    den1 = pw.tile([WHALF, H], F32, tag="den1")
                    nc.vector.affine_then_add(den1, Qsq, Ssq, 1.0, C1)  # d1+C1
                    t1 = pw.tile([WHALF, H], F32, tag="t1")
                    nc.vector.affine_then_add(t1, num1, As, -1.0, 0.0)  # As-num1
                    num2 = pw.tile([WHALF, H], F32, tag="num2")
                    nc.vector.affine_then_add(num2, Bs, t1, -1.0, 0.0)  # t1-Bs
                    t2 = pw.tile([WHALF, H], F32, tag="t2")
                    nc.vector.affine_then_add(t2, den1, As, -1.0, 0.0)  # As-den1
                    den2 = pw.tile([WHALF, H], F32, tag="den2")
                    nc.vector.affine_then_add(den2, Bs, t2, 1.0, 0.0)  # t2+Bs
                    den = pw.tile([WHALF, H], F32, tag="den")
                    nc.vector.tensor_mul(den, den1, den2)
                    r = pw.tile([WHALF, H], F32, tag="r")
                    nc.vector.reciprocal_approx_fast(out=r, in_=den)
                    num = pw.tile([WHALF, H], F32, tag="num")
                    nc.vector.tensor_mul(num, num1, num2)
                    prod = pw.tile([WHALF, H], F32, tag="prod")
                    nc.vector.scalar_tensor_tensor(
                        out=prod,
                        in0=num,
                        scalar=1.0,
                        in1=r,
                        op0=mybir.AluOpType.mult,
                        op1=mybir.AluOpType.mult,
                        accum_out=partials[:, 2 * p + m : 2 * p + m + 1],
                    )

            # ---- reduce partials columns -> [128, 1], DMA out ----
            red = accp.tile([128, 1], F32)
            nc.vector.reduce_sum(red, partials, axis=mybir.AxisListType.X)
            nc.gpsimd.dma_start(out=partials_out[:, :], in_=red)

    nc.finalize()
    return nc


def _get_nc():
    if "nc" not in _CACHE:
        _CACHE["nc"] = _build_nc()
    return _CACHE["nc"]


def _host_kl(img1, img2):
    """Host-side KLDiv branch value (only consumed when ssim > 0.75)."""
    x1 = img1.reshape(B, H * W).astype(np.float32)
    x2 = img2.reshape(B, H * W).astype(np.float32)

    def row_hist(x):
        mn = x.min(axis=1, keepdims=True)
        mx = x.max(axis=1, keepdims=True)
        width = mx - mn
        scaled = np.where(width > 0, (x - mn) * NBIN / width, 0.0)
        idx = np.clip(scaled.astype(np.int32), 0, NBIN - 1)
        h = np.zeros((B, NBIN), np.float32)
        for r in range(B):
            h[r] = np.bincount(idx[r], minlength=NBIN)
        return h

    def softmax(h):
        e = np.exp(h - h.max(axis=1, keepdims=True))
        return e / e.sum(axis=1, keepdims=True)

    p1 = softmax(row_hist(x1))
    p2 = softmax(row_hist(x2))
    return float(np.sum(np.exp(p2) * (p2 - p1)) / B)


def kernel(img1, img2, window):
    img1 = np.asarray(img1, dtype=np.float32)
    img2 = np.asarray(img2, dtype=np.float32)
    window = np.asarray(window, dtype=np.float32)

    # Recover the 1-D taps from the passed 2-D window (rows sum to g_i since
    # sum(g)=1), keeping the kernel faithful to the provided window input.
    g = window[0, 0].sum(axis=1)
    g = (g / g.sum()).astype(np.float32)
    bandA_hi, bandA_lo, bandB0, bandB1 = _make_bands(g)

    x = img1.reshape(B, H, W)
    y = img2.reshape(B, H, W)

    nc = _get_nc()
    in_maps = []
    for c in range(NCORES):
        sl = slice(c * PPC, (c + 1) * PPC)
        in_maps.append(
            {
                "img1": np.ascontiguousarray(x[sl]),
                "img2": np.ascontiguousarray(y[sl]),
                "bandA_hi": bandA_hi,
                "bandA_lo": bandA_lo,
                "bandB0": bandB0,
                "bandB1": bandB1,
            }
        )

    res = run_bass_kernel_spmd(nc, in_maps, core_ids=list(range(NCORES)))
    total = 0.0
    for c in range(NCORES):
        total += float(res.results[c]["partials"].sum())
    ssim = total / float(B * C * H * W)

    if ssim > 0.75:
        out = _host_kl(img1, img2) + 1.0 - ssim
    else:
        out = 1.0 - ssim
    return np.float32(out)


if __name__ == "__main__":
    rng = np.random.default_rng(0)
    i1 = rng.standard_normal((B, C, H, W), dtype=np.float32)
    i2 = rng.standard_normal((B, C, H, W), dtype=np.float32)
    g = _gauss_taps()
    w2 = np.outer(g, g).astype(np.float32)[None, None]
    print("out:", kernel(i1, i2, w2))
